# revision 24
# baseline (speedup 1.0000x reference)
"""DigitCaps routing kernel for TRN2 (8 NeuronCores, SPMD data-parallel over batch).

Problem: inputs [64, 4096, 8] f32, W [4096, 10, 8, 16] f32.
  u_hat[b,i,n,d] = sum_p inputs[b,i,p] * W[i,n,p,d]
  3 dynamic-routing iterations (softmax over n, weighted sum over i, squash,
  agreement update), output v [64, 10, 1, 16] f32.

Sharding: batch 64 -> 8 cores x 8 samples. W replicated (streamed once per core).

Per-core device layout (sigma = i'*8 + b, where i' = i mod 16, b = local sample):
  U  [128=(i'*8+b), 256 chunks * 160]  f32   u_hat, chunk k holds i in [16k,16k+16)
  L  [128, 256*10] f32                       routing logits

Key speed structure (vs the naive fp32 version):
- u_hat production in fp16 hi/lo pairs: u = xh*Wh + xh*Wl + xl*Wh (3 matmuls
  at 1cy/row instead of one fp32 matmul at 4cy/row). Verified numerically:
  final rel err 2.6e-4 (routing amplifies u_hat noise ~1e4x, so plain fp16 or
  fp32r matmuls fail; the hi/lo pair keeps ~22 mantissa bits).
- s0 (uniform-c weighted sum) costs no PE time: GPSIMD tensor_reduce
  accumulates sum_i u_hat from the U copies while produce runs; one small
  matmul folds partitions at the end (replaces 256 fp32 matmuls).
- logit update (b += sum_d u*v) via scalar_tensor_tensor (TensorScalarPtr
  runs 2x_2p = 0.52ns/elem for fp32 SBUF operands, vs TensorTensor 1x) and a
  pairwise d-reduction tree (also 2x) instead of TensorReduce (modeless 1x).
- s-accumulation matmuls stay fp32: c needs >=17 mantissa bits (bf16/fp16 c
  fails: 4.5e-2), and an fp32r cast of U to 12 bits also fails (3.6e-2).
- squash: v = s*|s|/(1+s^2) - no Sqrt, whole kernel stays on one ACT table
  set (exp_and_others has Abs), saving ~2.7us per table swap.
"""

from contextlib import ExitStack

import numpy as np

import concourse.bass as bass
import concourse.tile as tile
from concourse import bacc, mybir
from concourse.tile import TileContext

AF = mybir.ActivationFunctionType
ALU = mybir.AluOpType

N_CORES = 8
B_FULL = 64
I_FULL = 4096
P_DIM = 8          # Din
N_CAP = 10
D_CAP = 16
ND = N_CAP * D_CAP  # 160
EPS = 1e-7
ROUTING_ITERS = 3

F32 = mybir.dt.float32
F16 = mybir.dt.float16


def build_nc(I_dim=I_FULL, b_shard=8, phases="all", repeat=1):
    """Build the single-core Bass program (SPMD: same program on all cores).

    phases: "all" | "prod" (produce + s0 + squash) | "it1" (.. + first
            s-iteration) - for cost attribution.
    repeat: run the whole pipeline N times back-to-back (for wall-clock
            timing: the per-repeat delta cancels dispatch overhead).
    """
    CH = I_dim // 16          # chunks of 16 capsules
    SUPC = 4                  # chunks per DMA superchunk
    SUP = CH // SUPC
    GRP = min(16, CH)         # chunks per group for DVE staging
    NG = CH // GRP

    nc = bacc.Bacc(dynamic_dma_scratch_size=1024)

    # w2p[(k*128+q), (h*160 + n*16+d)] = fp16 hi/lo pair of W (h=0 hi, h=1 lo)
    w2p_d = nc.dram_tensor("w2p", [CH * 128, 2 * ND], F16, kind="ExternalInput")
    # xpk[q=(i'*8+p), h*CH*8 + k*8+b] = fp16 hi/lo pair of x
    xpkp_d = nc.dram_tensor("xpkp", [128, 2 * CH * 8], F16, kind="ExternalInput")
    # xmask[q, m=(i2*8+b)] = 1 if i2 == q//8 else 0 (block-diagonal selector)
    xmask_d = nc.dram_tensor("xmask", [128, 128], F16, kind="ExternalInput")
    mask0_d = nc.dram_tensor("mask0", [128, 8], F32, kind="ExternalInput")
    maskc_d = nc.dram_tensor("maskc", [128, 80], F32, kind="ExternalInput")
    e8_d = nc.dram_tensor("e8", [8, 128], F32, kind="ExternalInput")
    # maskd[(n*8+b), n'*16+d] = (n'==n); selb[(n*8+b), b'] = (b'==b)
    maskd_d = nc.dram_tensor("maskd", [128, ND], F32, kind="ExternalInput")
    selb_d = nc.dram_tensor("selb", [128, 8], F32, kind="ExternalInput")
    out_d = nc.dram_tensor("out", [b_shard, ND], F32, kind="ExternalOutput")
    udbg_d = (nc.dram_tensor("udbg", [128, CH * ND], F32, kind="ExternalOutput")
              if phases == "proddbg" else None)
    adbg_d = (nc.dram_tensor("adbg", [128, 4 * ND], F32, kind="ExternalOutput")
              if phases == "proddbg" else None)

    with TileContext(nc) as tc, ExitStack() as ctx:
        # ---- pools ----
        pU = ctx.enter_context(tc.tile_pool(name="U", bufs=1))
        pL = ctx.enter_context(tc.tile_pool(name="L", bufs=1))
        pconst = ctx.enter_context(tc.tile_pool(name="const", bufs=1))
        pw2 = ctx.enter_context(tc.tile_pool(name="w2", bufs=3))
        pxa = ctx.enter_context(tc.tile_pool(name="xa", bufs=2))
        psm = ctx.enter_context(tc.tile_pool(name="sm", bufs=2))
        pprod = ctx.enter_context(tc.tile_pool(name="prod", bufs=2))
        pca = ctx.enter_context(tc.tile_pool(name="ca", bufs=2))
        psmall = ctx.enter_context(tc.tile_pool(name="small", bufs=1))
        pacc = ctx.enter_context(tc.tile_pool(name="acc", bufs=1))
        ps0 = ctx.enter_context(tc.tile_pool(name="s0", bufs=2))
        ppsP = ctx.enter_context(tc.tile_pool(name="psP", bufs=4, space="PSUM"))
        ppsS = ctx.enter_context(tc.tile_pool(name="psS", bufs=1, space="PSUM"))
        ppsV = ctx.enter_context(tc.tile_pool(name="psV", bufs=1, space="PSUM"))
        ppsF = ctx.enter_context(tc.tile_pool(name="psF", bufs=1, space="PSUM"))

        # ---- persistent tiles ----
        U = pU.tile([128, CH * ND], F32)
        L = pL.tile([128, CH * N_CAP], F32)
        xpkp_sb = pconst.tile([128, 2 * CH * 8], F16)
        xmask_sb = pconst.tile([128, 128], F16)
        mask0_sb = pconst.tile([128, 8], F32)
        maskc_sb = pconst.tile([128, 80], F32)
        e8_sb = pconst.tile([8, 128], F32)
        maskd_sb = pconst.tile([128, ND], F32)
        selb_sb = pconst.tile([128, 8], F32)

        nc.sync.dma_start(xpkp_sb[:], xpkp_d[:])
        nc.sync.dma_start(xmask_sb[:], xmask_d[:])
        nc.sync.dma_start(mask0_sb[:], mask0_d[:])
        nc.sync.dma_start(maskc_sb[:], maskc_d[:])
        nc.sync.dma_start(e8_sb[:], e8_d[:])
        nc.sync.dma_start(maskd_sb[:], maskd_d[:])
        nc.sync.dma_start(selb_sb[:], selb_d[:])
        nc.vector.memset(L[:], 0.0)

        # ---- phase A: u_hat production (fp16 hi/lo 3-term) + s0 accumulation
        # per superchunk: DMA w2 pair, build block-diagonal X tiles on-chip
        # (broadcast xpk hi/lo against xmask), then per chunk 3 accumulating
        # matmuls; 3 chunk outputs share one PSUM bank -> one batched copy.
        # GPSIMD reduces copied U group-slices into the s0 accumulator.
        w2_r = w2p_d.rearrange("(s c p) f -> s p c f", c=SUPC, p=128)
        xpk_r = xpkp_sb.rearrange("p (h s c b) -> p h s c b", h=2, c=SUPC, b=8)

        NACC = 4   # accs[0:2]: pair-sum accumulators; accs[2:4]: singles
        accs = []
        for j in range(NACC):
            acc_j = pacc.tile([128, ND], F32, tag=f"acc{j}")
            accs.append(acc_j)

        def produce():
          ps = None
          for s in range(SUP):
            w2b = pw2.tile([128, SUPC * 2 * ND], F16)
            nc.sync.dma_start(
                w2b.rearrange("p (c f) -> p c f", c=SUPC), w2_r[s])
            w2v = w2b.rearrange("p (c h f) -> p c h f", c=SUPC, h=2)
            xab = pxa.tile([128, 2 * SUPC * 128], F16)
            xav = xab.rearrange("p (h c i b) -> p h c i b", h=2, c=SUPC, b=8)
            for h in range(2):
                # all-fp16 tensor_tensor: 2x_1p DVE mode
                nc.vector.tensor_tensor(
                    xav[:, h],
                    xpk_r[:, h, s].unsqueeze(2).to_broadcast([128, SUPC, 16, 8]),
                    xmask_sb.rearrange("p (i b) -> p i b", b=8)
                        .unsqueeze(1).to_broadcast([128, SUPC, 16, 8]),
                    ALU.mult,
                )
            for c in range(SUPC):
                k = s * SUPC + c
                j = k % 3
                if j == 0:
                    ps = ppsP.tile([128, 3 * ND], F32)
                xh = xav[:, 0, c].rearrange("p i b -> p (i b)")
                xl = xav[:, 1, c].rearrange("p i b -> p (i b)")
                dst = ps[:, j * ND:(j + 1) * ND]
                nc.tensor.matmul(dst, xh, w2v[:, c, 0], start=True, stop=False)
                nc.tensor.matmul(dst, xh, w2v[:, c, 1], start=False, stop=False)
                nc.tensor.matmul(dst, xl, w2v[:, c, 0], start=False, stop=True)
                if j == 2 or k == CH - 1:
                    lo = k - j
                    nc.scalar.copy(
                        U[:, lo * ND:(k + 1) * ND], ps[:, 0:(j + 1) * ND])
                    # s0 accumulation on the freshly copied chunks: POOL sums
                    # the first pair, DVE folds into rotating partial accs
                    g3 = k // 3
                    if j >= 1:
                        t01 = ps0.tile([128, ND], F32, tag="s0a")
                        nc.gpsimd.tensor_tensor(
                            t01[:], U[:, lo * ND:(lo + 1) * ND],
                            U[:, (lo + 1) * ND:(lo + 2) * ND], ALU.add)
                        a0 = accs[g3 % 2]
                        if g3 < 2:
                            nc.vector.tensor_copy(a0[:], t01[:])
                        else:
                            nc.vector.tensor_tensor(a0[:], a0[:], t01[:],
                                                    ALU.add)
                    if j != 1:
                        a1 = accs[2 + g3 % 2]
                        lastc = U[:, k * ND:(k + 1) * ND]
                        if g3 < 2:
                            nc.vector.tensor_copy(a1[:], lastc)
                        else:
                            nc.vector.tensor_tensor(a1[:], a1[:], lastc,
                                                    ALU.add)

        # ---- helpers ----
        def squash(in_ap, scale, copy_in=True):
            """v = s*|s|/(1+s^2) (== reference squash up to its eps guard;
            max deviation ~1e-4). No Sqrt -> one ACT table set."""
            if copy_in:
                s_sb = psmall.tile([8, ND], F32, tag="sq_s")
                nc.scalar.mul(s_sb[:], in_ap, scale)
            else:
                s_sb = in_ap  # already an SBUF tile, scale must be 1
            sq = psmall.tile([8, ND], F32, tag="sq_sq")
            nc.vector.tensor_mul(sq[:], s_sb[:], s_sb[:])
            nc.vector.tensor_scalar_add(sq[:], sq[:], 1.0)
            nc.vector.reciprocal(sq[:], sq[:])
            ab = psmall.tile([8, ND], F32, tag="sq_rt")
            nc.scalar.activation(ab[:], s_sb[:], AF.Abs)
            nc.vector.tensor_mul(ab[:], ab[:], sq[:])
            v_sb = psmall.tile([8, ND], F32, tag="sq_v")
            nc.vector.tensor_mul(v_sb[:], ab[:], s_sb[:])
            return v_sb

        def s_uniform():
            """s0 = 0.1 * sum_i u_hat: fold the 4 partial accs across
            partitions via one accumulating matmul chain (0.1 in mask0)."""
            s0_ps = ppsS.tile([8, ND], F32, tag="s_acc")
            for j in range(NACC):
                nc.tensor.matmul(s0_ps[:], mask0_sb[:], accs[j][:],
                                 start=(j == 0), stop=(j == NACC - 1))
            return s0_ps

        def broadcast_v(v_sb):
            """v [8,160] -> vf [128,160] (replicated per sample block)."""
            vf_ps = ppsV.tile([128, ND], F32)
            nc.tensor.matmul(vf_ps[:], e8_sb[:], v_sb[:], start=True, stop=True)
            vf = psmall.tile([128, ND], F32, tag="vf")
            nc.vector.tensor_copy(vf[:], vf_ps[:])
            return vf

        GL = 8   # chunks per logit subgroup (pr half-size -> bufs=2 fits)

        def logit_update_group(gl, vf):
            """L[sigma, (k,n)] += sum_d U[sigma,(k,n,d)] * v, subgroup gl
            of GL chunks. fp32 throughout (fp16 anywhere in the logit path
            fails: measured 2.3e-2..0.19 rel err vs the 2e-2 gate).
            Multiply on DVE; pairwise-add d-tree in place inside pr with the
            wide rounds on GPSIMD (otherwise idle), tail + L add on DVE."""
            Lv = L.rearrange("p (a x) -> p a x", x=GL * N_CAP)
            pr = pprod.tile([128, GL * ND], F32, tag="pr")
            nc.vector.tensor_tensor(
                pr.rearrange("p (k f) -> p k f", k=GL),
                U[:, gl * GL * ND:(gl + 1) * GL * ND]
                    .rearrange("p (k f) -> p k f", k=GL),
                vf[:].unsqueeze(1).to_broadcast([128, GL, ND]),
                ALU.mult,
            )
            pv = pr.rearrange("p (a d) -> p a d", d=D_CAP)  # a = GL*N_CAP
            for w in (8, 4, 2):
                nc.gpsimd.tensor_tensor(
                    pv[:, :, 0:w], pv[:, :, 0:w], pv[:, :, w:2 * w], ALU.add)
            nc.vector.tensor_tensor(
                pv[:, :, 0:1], pv[:, :, 0:1], pv[:, :, 1:2], ALU.add)
            nc.vector.tensor_tensor(
                Lv[:, gl, :], Lv[:, gl, :],
                pv[:, :, 0:1].rearrange("p a d -> p (a d)"),
                ALU.add)

        def softmax_smm_group(g, s_ps):
            """Group-local softmax + C_all build + the s accumulation MMs."""
            Lg = L.rearrange("p (g k n) -> p g k n", k=GRP, n=N_CAP)
            ee = psm.tile([128, GRP * N_CAP], F32, tag="ee")
            nc.scalar.activation(
                ee[:], Lg[:, g].rearrange("p k n -> p (k n)"), AF.Exp)
            eev = ee.rearrange("p (k n) -> p k n", n=N_CAP)
            den = psm.tile([128, GRP], F32, tag="den")
            nc.vector.tensor_reduce(
                den[:], eev, axis=mybir.AxisListType.X, op=ALU.add)
            rr = psm.tile([128, GRP], F32, tag="rr")
            nc.vector.reciprocal(rr[:], den[:])
            cc = psm.tile([128, GRP * N_CAP], F32, tag="cc")
            nc.vector.tensor_tensor(
                cc.rearrange("p (k n) -> p k n", n=N_CAP), eev,
                rr[:].unsqueeze(2).to_broadcast([128, GRP, N_CAP]),
                ALU.mult,
            )
            ca = pca.tile([128, GRP * 80], F32)
            nc.vector.tensor_tensor(
                ca.rearrange("p (k n b) -> p k n b", k=GRP, b=8),
                cc.rearrange("p (k n) -> p k n", n=N_CAP)
                    .unsqueeze(3).to_broadcast([128, GRP, N_CAP, 8]),
                maskc_sb.rearrange("p (n b) -> p n b", b=8)
                    .unsqueeze(1).to_broadcast([128, GRP, N_CAP, 8]),
                ALU.mult,
            )
            for kk in range(GRP):
                k = g * GRP + kk
                nc.tensor.matmul(
                    s_ps[:],
                    ca[:, kk * 80:(kk + 1) * 80],
                    U[:, k * ND:(k + 1) * ND],
                    start=(k == 0), stop=(k == CH - 1),
                )

        def s_iteration(v_prev=None):
            """If v_prev is given, fuse its logit-update per group with this
            iteration's softmax + s-matmuls (the PE chases the DVE)."""
            vf = broadcast_v(v_prev) if v_prev is not None else None
            s_ps = ppsS.tile([80, ND], F32, tag="s_acc")
            sub = GRP // GL
            fill_ps = ppsF.tile([8, ND], F32, tag="fill")
            for g in range(NG):
                if vf is not None:
                    for q in range(sub):
                        logit_update_group(g * sub + q, vf)
                softmax_smm_group(g, s_ps)
                # PE keep-warm fillers: the DVE/POOL chain for group g+1 takes
                # ~7us while the PE burst is ~5us; dummy matmuls bridge the gap
                # so the PE p-state stays at peak (idle >100ns drops matmuls to
                # half rate for the next 3us of execution)
                for _ in range(6):
                    nc.tensor.matmul(fill_ps[:], mask0_sb[:], accs[0][:],
                                     start=True, stop=True)
            return s_ps

        def extract_diag(s_ps):
            """[80,160] psum -> [8,160]: mask away the n-offdiagonal blocks,
            then one selection matmul folds (n*8+b) rows onto b rows."""
            sm = psmall.tile([80, ND], F32, tag="vf")
            nc.vector.tensor_tensor(sm[:], s_ps[:], maskd_sb[0:80, :],
                                    ALU.mult)
            s8_ps = ppsS.tile([8, ND], F32, tag="s_ex")
            nc.tensor.matmul(s8_ps[:], selb_sb[0:80, :], sm[:], start=True,
                             stop=True)
            s_sb = psmall.tile([8, ND], F32, tag="sq_s")
            nc.scalar.copy(s_sb[:], s8_ps[:])
            return s_sb

        # ---- routing (logit-update of iter t fused into s-iteration t+1) ----
        for rep in range(repeat):
            if rep > 0:
                nc.vector.memset(L[:], 0.0)
            produce()
            if phases in ("prod", "proddbg"):
                v_sb = squash(s_uniform()[:], 1.0)
            elif phases == "it1":
                v_sb = squash(s_uniform()[:], 1.0)
                v_sb = squash(extract_diag(s_iteration(v_sb))[:], 1.0,
                              copy_in=False)
            else:
                # the 1/N is baked into mask0
                v_sb = squash(s_uniform()[:], 1.0)
                for it in range(1, ROUTING_ITERS):
                    s_sb = extract_diag(s_iteration(v_sb))
                    v_sb = squash(s_sb[:], 1.0, copy_in=False)

            nc.sync.dma_start(out_d[:], v_sb[:])
            if udbg_d is not None:
                nc.sync.dma_start(udbg_d[:], U[:])
                for j in range(4):
                    nc.sync.dma_start(
                        adbg_d[:, j * ND:(j + 1) * ND], accs[j][:])

    nc.compile()
    if not nc.is_finalized():
        nc.finalize()
    return nc


# ------------------------- host-side data prep -------------------------

def prep_core_inputs(x_shard, I_dim=I_FULL):
    """Per-core xpk fp16 hi/lo pair from x_shard [8, I, 8] f32."""
    CH = I_dim // 16
    b_shard = x_shard.shape[0]
    assert b_shard == 8

    # xs[b, k, i', p] -> xpk[(i'*8+p), k*8+b]
    xs = x_shard.reshape(b_shard, CH, 16, P_DIM)
    xpk = np.ascontiguousarray(
        np.transpose(xs, (2, 3, 1, 0)).reshape(128, CH * 8))
    xh = xpk.astype(np.float16)
    xl = (xpk - xh.astype(np.float32)).astype(np.float16)
    return {"xpkp": np.ascontiguousarray(np.concatenate([xh, xl], axis=1))}


def prep_shared_inputs(W_np):
    # w2[(i*8+p), n*16+d] = W[i, n, p, d]; stored as fp16 hi/lo pair
    w2 = np.ascontiguousarray(
        np.transpose(W_np, (0, 2, 1, 3)).reshape(-1, ND).astype(np.float32))
    w2h = w2.astype(np.float16)
    w2l = (w2 - w2h.astype(np.float32)).astype(np.float16)
    w2p = np.ascontiguousarray(np.concatenate([w2h, w2l], axis=1))

    # mask0[sigma, b'] = 0.1 * (b' == b(sigma));  sigma = i'*8+b
    # maskc[sigma, n*8+b'] = (b' == b(sigma)); s-matmul output partition = n*8+b
    # xmask[q=(i'*8+p), i2*8+b] = (i2 == i')
    mask0 = np.zeros((128, 8), dtype=np.float32)
    maskc = np.zeros((128, 80), dtype=np.float32)
    e8 = np.zeros((8, 128), dtype=np.float32)
    xmask = np.zeros((128, 128), dtype=np.float16)
    maskd = np.zeros((128, ND), dtype=np.float32)
    selb = np.zeros((128, 8), dtype=np.float32)
    for ip in range(16):
        for b in range(8):
            sig = ip * 8 + b
            mask0[sig, b] = 0.1
            maskc[sig, b::8] = 1.0
            e8[b, sig] = 1.0
    for ii in range(16):
        for p in range(P_DIM):
            xmask[ii * 8 + p, ii * 8:(ii + 1) * 8] = 1.0
    for n in range(N_CAP):
        for b in range(8):
            maskd[n * 8 + b, n * D_CAP:(n + 1) * D_CAP] = 1.0
            selb[n * 8 + b, b] = 1.0
    return {"w2p": w2p, "mask0": mask0, "maskc": maskc, "e8": e8,
            "xmask": xmask, "maskd": maskd, "selb": selb}


_NC_CACHE = {}
LAST_RESULT = None  # BassKernelResults of the most recent kernel() call


def _get_nc(I_dim=I_FULL):
    if I_dim not in _NC_CACHE:
        _NC_CACHE[I_dim] = build_nc(I_dim)
    return _NC_CACHE[I_dim]


def kernel(inputs: np.ndarray, W: np.ndarray, trace: bool = False) -> np.ndarray:
    global LAST_RESULT
    from concourse.bass_utils import run_bass_kernel_spmd

    inputs = np.asarray(inputs, dtype=np.float32)
    W = np.asarray(W, dtype=np.float32)
    B, I_dim, _ = inputs.shape

    nc = _get_nc(I_dim)
    shared = prep_shared_inputs(W)

    in_maps = []
    bs = B // N_CORES
    for c in range(N_CORES):
        m = dict(shared)
        m.update(prep_core_inputs(inputs[c * bs:(c + 1) * bs], I_dim))
        in_maps.append(m)

    res = run_bass_kernel_spmd(nc, in_maps, list(range(N_CORES)), trace=trace)
    LAST_RESULT = res
    outs = [res.results[c]["out"] for c in range(N_CORES)]
    v = np.concatenate(outs, axis=0)          # [64, 160]
    v = v.reshape(B, N_CAP, D_CAP)[:, :, None, :]   # [64, 10, 1, 16]
    return v.astype(np.float32)


# revision 32
# speedup vs baseline: 1.4170x; 1.4170x over previous
"""DigitCaps routing kernel for TRN2 (8 NeuronCores, SPMD data-parallel over batch).

Problem: inputs [64, 4096, 8] f32, W [4096, 10, 8, 16] f32.
  u_hat[b,i,n,d] = sum_p inputs[b,i,p] * W[i,n,p,d]
  3 dynamic-routing iterations (softmax over n, weighted sum over i, squash,
  agreement update), output v [64, 10, 1, 16] f32.

Sharding: batch 64 -> 8 cores x 8 samples. W replicated (streamed once per core).

Per-core device layout (sigma = i'*8 + b, where i' = i mod 16, b = local sample):
  U  [128=(i'*8+b), 256 chunks * 160]  f32   u_hat, chunk k holds i in [16k,16k+16)
  L  [128, 256*10] f32                       routing logits

Key speed structure (HW-measured; the cost model misleads in places):
- produce: one fp32 matmul per chunk (block-diagonal x lhsT, fp16 mask
  build replaced by fp32). HW fp32 matmul runs ~2x the cost model's
  4cy/row, so the fp16 hi/lo 3-term variant (fp16_produce=True) measured
  ~90us SLOWER end to end despite fewer model-cycles (extra ldweights +
  2 more matmuls/chunk).
- s0 = sum_i u_hat: DVE pair-adds fold freshly copied chunks into 4
  rotating partial accumulators during produce (aligned to the 3-chunk
  PSUM copy groups); one small matmul chain folds partitions at the end
  (replaces 256 s_uniform matmuls = 68us of PE). s0_pe (extra per-chunk
  PE matmul, interleaved accumulation groups) measured far slower on HW.
- logit update (b += sum_d u*v): DVE TensorTensor mult + per-group
  d-tree / TensorReduce, all fp32. GPSIMD offload measured 67us slower
  on HW than modeled; all-fp16 logit paths fail accuracy (measured
  2.3e-2..0.19 vs the 2e-2 gate - routing amplifies per-i noise ~1e4x,
  so u_hat/logit math needs >=16 mantissa bits and c needs >=17).
- s-accumulation matmuls stay fp32 (same precision argument).
- PE keep-warm filler matmuls between iteration groups hold the p-state.
- squash: v = s*|s|/(1+s^2) - no Sqrt, whole kernel stays on one ACT table
  set (exp_and_others has Abs), saving ~2.7us per table swap.
"""

from contextlib import ExitStack

import numpy as np

import concourse.bass as bass
import concourse.tile as tile
from concourse import bacc, mybir
from concourse.tile import TileContext

AF = mybir.ActivationFunctionType
ALU = mybir.AluOpType

N_CORES = 8
B_FULL = 64
I_FULL = 4096
P_DIM = 8          # Din
N_CAP = 10
D_CAP = 16
ND = N_CAP * D_CAP  # 160
EPS = 1e-7
ROUTING_ITERS = 3

F32 = mybir.dt.float32
F16 = mybir.dt.float16


def build_nc(I_dim=I_FULL, b_shard=8, phases="all", repeat=1,
             use_pool=False, fp16_produce=False, tree_reduce=False,
             gl=8, fillers=6, s0_pe=False):
    """Build the single-core Bass program (SPMD: same program on all cores).

    phases: "all" | "prod" (produce + s0 + squash) | "it1" (.. + first
            s-iteration) - for cost attribution.
    repeat: run the whole pipeline N times back-to-back (for wall-clock
            timing: the per-repeat delta cancels dispatch overhead).
    use_pool: offload s0 pair-sums and the logit d-tree to GPSIMD. Set
            False to keep everything on DVE (GPSIMD cost on real HW is
            suspect vs the cost model).
    """
    CH = I_dim // 16          # chunks of 16 capsules
    SUPC = 4                  # chunks per DMA superchunk
    SUP = CH // SUPC
    GRP = min(16, CH)         # chunks per group for DVE staging
    if s0_pe:
        fillers = 0           # s0raw psum bank takes the filler bank
    NG = CH // GRP

    nc = bacc.Bacc(dynamic_dma_scratch_size=1024)

    NH = 2 if fp16_produce else 1   # fp16 hi/lo pair vs plain fp32
    FPROD = F16 if fp16_produce else F32
    # w2p[(k*128+q), (h*160 + n*16+d)]: h=0 hi, h=1 lo (fp16) or h=0 (fp32)
    w2p_d = nc.dram_tensor("w2p", [CH * 128, NH * ND], FPROD,
                           kind="ExternalInput")
    # xpk[q=(i'*8+p), h*CH*8 + k*8+b]
    xpkp_d = nc.dram_tensor("xpkp", [128, NH * CH * 8], FPROD,
                            kind="ExternalInput")
    # xmask[q, m=(i2*8+b)] = 1 if i2 == q//8 else 0 (block-diagonal selector)
    xmask_d = nc.dram_tensor("xmask", [128, 128], FPROD, kind="ExternalInput")
    mask0_d = nc.dram_tensor("mask0", [128, 8], F32, kind="ExternalInput")
    maskc_d = nc.dram_tensor("maskc", [128, 80], F32, kind="ExternalInput")
    e8_d = nc.dram_tensor("e8", [8, 128], F32, kind="ExternalInput")
    # maskd[(n*8+b), n'*16+d] = (n'==n); selb[(n*8+b), b'] = (b'==b)
    maskd_d = nc.dram_tensor("maskd", [128, ND], F32, kind="ExternalInput")
    selb_d = nc.dram_tensor("selb", [128, 8], F32, kind="ExternalInput")
    out_d = nc.dram_tensor("out", [b_shard, ND], F32, kind="ExternalOutput")
    udbg_d = (nc.dram_tensor("udbg", [128, CH * ND], F32, kind="ExternalOutput")
              if phases == "proddbg" else None)
    adbg_d = (nc.dram_tensor("adbg", [128, 4 * ND], F32, kind="ExternalOutput")
              if phases == "proddbg" else None)

    with TileContext(nc) as tc, ExitStack() as ctx:
        # ---- pools ----
        pU = ctx.enter_context(tc.tile_pool(name="U", bufs=1))
        pL = ctx.enter_context(tc.tile_pool(name="L", bufs=1))
        pconst = ctx.enter_context(tc.tile_pool(name="const", bufs=1))
        pw2 = ctx.enter_context(tc.tile_pool(name="w2", bufs=3))
        pxa = ctx.enter_context(tc.tile_pool(name="xa", bufs=2))
        psm = ctx.enter_context(tc.tile_pool(name="sm", bufs=2))
        pprod = ctx.enter_context(tc.tile_pool(name="prod", bufs=2))
        pca = ctx.enter_context(tc.tile_pool(name="ca", bufs=2))
        psmall = ctx.enter_context(tc.tile_pool(name="small", bufs=1))
        pacc = ctx.enter_context(tc.tile_pool(name="acc", bufs=1))
        ps0 = ctx.enter_context(tc.tile_pool(name="s0", bufs=2))
        ppsP = ctx.enter_context(tc.tile_pool(name="psP", bufs=4, space="PSUM"))
        ppsS = ctx.enter_context(tc.tile_pool(name="psS", bufs=1, space="PSUM"))
        ppsV = ctx.enter_context(tc.tile_pool(name="psV", bufs=1, space="PSUM"))
        ppsF = ctx.enter_context(tc.tile_pool(name="psF", bufs=1, space="PSUM"))

        # ---- persistent tiles ----
        U = pU.tile([128, CH * ND], F32)
        L = pL.tile([128, CH * N_CAP], F32)
        xpkp_sb = pconst.tile([128, NH * CH * 8], FPROD)
        xmask_sb = pconst.tile([128, 128], FPROD)
        mask0_sb = pconst.tile([128, 8], F32)
        maskc_sb = pconst.tile([128, 80], F32)
        e8_sb = pconst.tile([8, 128], F32)
        maskd_sb = pconst.tile([128, ND], F32)
        selb_sb = pconst.tile([128, 8], F32)

        nc.sync.dma_start(xpkp_sb[:], xpkp_d[:])
        nc.sync.dma_start(xmask_sb[:], xmask_d[:])
        nc.sync.dma_start(mask0_sb[:], mask0_d[:])
        nc.sync.dma_start(maskc_sb[:], maskc_d[:])
        nc.sync.dma_start(e8_sb[:], e8_d[:])
        nc.sync.dma_start(maskd_sb[:], maskd_d[:])
        nc.sync.dma_start(selb_sb[:], selb_d[:])
        nc.vector.memset(L[:], 0.0)

        # ---- phase A: u_hat production (fp16 hi/lo 3-term) + s0 accumulation
        # per superchunk: DMA w2 pair, build block-diagonal X tiles on-chip
        # (broadcast xpk hi/lo against xmask), then per chunk 3 accumulating
        # matmuls; 3 chunk outputs share one PSUM bank -> one batched copy.
        # GPSIMD reduces copied U group-slices into the s0 accumulator.
        w2_r = w2p_d.rearrange("(s c p) f -> s p c f", c=SUPC, p=128)
        xpk_r = xpkp_sb.rearrange("p (h s c b) -> p h s c b", h=NH, c=SUPC,
                                  b=8)

        NACC = 4   # accs[0:2]: pair-sum accumulators; accs[2:4]: singles
        accs = []
        if not s0_pe:
            for j in range(NACC):
                acc_j = pacc.tile([128, ND], F32, tag=f"acc{j}")
                accs.append(acc_j)

        def produce():
          ps = None
          s0raw_ps = None
          if s0_pe:
              s0raw_ps = ppsF.tile([8, ND], F32, tag="s0raw")
          for s in range(SUP):
            w2b = pw2.tile([128, SUPC * NH * ND], FPROD)
            nc.sync.dma_start(
                w2b.rearrange("p (c f) -> p c f", c=SUPC), w2_r[s])
            w2v = w2b.rearrange("p (c h f) -> p c h f", c=SUPC, h=NH)
            xab = pxa.tile([128, NH * SUPC * 128], FPROD)
            xav = xab.rearrange("p (h c i b) -> p h c i b", h=NH, c=SUPC, b=8)
            for h in range(NH):
                # all-fp16 tensor_tensor: 2x_1p DVE mode
                nc.vector.tensor_tensor(
                    xav[:, h],
                    xpk_r[:, h, s].unsqueeze(2).to_broadcast([128, SUPC, 16, 8]),
                    xmask_sb.rearrange("p (i b) -> p i b", b=8)
                        .unsqueeze(1).to_broadcast([128, SUPC, 16, 8]),
                    ALU.mult,
                )
            for c in range(SUPC):
                k = s * SUPC + c
                j = k % 3
                if j == 0:
                    ps = ppsP.tile([128, 3 * ND], F32)
                xh = xav[:, 0, c].rearrange("p i b -> p (i b)")
                dst = ps[:, j * ND:(j + 1) * ND]
                if fp16_produce:
                    xl = xav[:, 1, c].rearrange("p i b -> p (i b)")
                    nc.tensor.matmul(dst, xh, w2v[:, c, 0], start=True,
                                     stop=False)
                    nc.tensor.matmul(dst, xh, w2v[:, c, 1], start=False,
                                     stop=False)
                    nc.tensor.matmul(dst, xl, w2v[:, c, 0], start=False,
                                     stop=True)
                else:
                    nc.tensor.matmul(dst, xh, w2v[:, c, 0], start=True,
                                     stop=True)
                if s0_pe:
                    xpk_k = xpk_r[:, 0, s, c]   # [128, 8] raw x (unmasked)
                    nc.tensor.matmul(s0raw_ps[:], xpk_k, w2v[:, c, 0],
                                     start=(k == 0), stop=(k == CH - 1))
                    if fp16_produce:
                        nc.tensor.matmul(s0raw_ps[:], xpk_k, w2v[:, c, 1],
                                         start=False, stop=False,
                                         skip_group_check=True)
                        nc.tensor.matmul(s0raw_ps[:],
                                         xpk_r[:, 1, s, c], w2v[:, c, 0],
                                         start=False, stop=False,
                                         skip_group_check=True)
                if j == 2 or k == CH - 1:
                    lo = k - j
                    nc.scalar.copy(
                        U[:, lo * ND:(k + 1) * ND], ps[:, 0:(j + 1) * ND])
                    # s0 accumulation on the freshly copied chunks: POOL sums
                    # the first pair, DVE folds into rotating partial accs
                    g3 = k // 3
                    if s0_pe:
                        pass
                    elif j >= 1:
                        t01 = ps0.tile([128, ND], F32, tag="s0a")
                        peng = nc.gpsimd if use_pool else nc.vector
                        peng.tensor_tensor(
                            t01[:], U[:, lo * ND:(lo + 1) * ND],
                            U[:, (lo + 1) * ND:(lo + 2) * ND], ALU.add)
                        a0 = accs[g3 % 2]
                        if g3 < 2:
                            nc.vector.tensor_copy(a0[:], t01[:])
                        else:
                            nc.vector.tensor_tensor(a0[:], a0[:], t01[:],
                                                    ALU.add)
                    if (not s0_pe) and j != 1:
                        a1 = accs[2 + g3 % 2]
                        lastc = U[:, k * ND:(k + 1) * ND]
                        if g3 < 2:
                            nc.vector.tensor_copy(a1[:], lastc)
                        else:
                            nc.vector.tensor_tensor(a1[:], a1[:], lastc,
                                                    ALU.add)
          return s0raw_ps

        # ---- helpers ----
        def squash(in_ap, scale, copy_in=True):
            """v = s*|s|/(1+s^2) (== reference squash up to its eps guard;
            max deviation ~1e-4). No Sqrt -> one ACT table set."""
            if copy_in:
                s_sb = psmall.tile([8, ND], F32, tag="sq_s")
                nc.scalar.mul(s_sb[:], in_ap, scale)
            else:
                s_sb = in_ap  # already an SBUF tile, scale must be 1
            sq = psmall.tile([8, ND], F32, tag="sq_sq")
            nc.vector.tensor_mul(sq[:], s_sb[:], s_sb[:])
            nc.vector.tensor_scalar_add(sq[:], sq[:], 1.0)
            nc.vector.reciprocal(sq[:], sq[:])
            ab = psmall.tile([8, ND], F32, tag="sq_rt")
            nc.scalar.activation(ab[:], s_sb[:], AF.Abs)
            nc.vector.tensor_mul(ab[:], ab[:], sq[:])
            v_sb = psmall.tile([8, ND], F32, tag="sq_v")
            nc.vector.tensor_mul(v_sb[:], ab[:], s_sb[:])
            return v_sb

        def s_uniform(s0raw_ps):
            """s0 = 0.1 * sum_i u_hat. s0_pe: the PE already accumulated
            sum_i u_hat in s0raw_ps (0.1 applied at squash). Otherwise fold
            the 4 partial accs across partitions (0.1 baked into mask0)."""
            if s0_pe:
                return s0raw_ps
            s0_ps = ppsS.tile([8, ND], F32, tag="s_acc")
            for j in range(NACC):
                nc.tensor.matmul(s0_ps[:], mask0_sb[:], accs[j][:],
                                 start=(j == 0), stop=(j == NACC - 1))
            return s0_ps

        def broadcast_v(v_sb):
            """v [8,160] -> vf [128,160] (replicated per sample block)."""
            vf_ps = ppsV.tile([128, ND], F32)
            nc.tensor.matmul(vf_ps[:], e8_sb[:], v_sb[:], start=True, stop=True)
            vf = psmall.tile([128, ND], F32, tag="vf")
            nc.vector.tensor_copy(vf[:], vf_ps[:])
            return vf

        GL = min(gl, GRP)  # chunks per logit subgroup

        def logit_update_group(gli, vf):
            """L[sigma, (k,n)] += sum_d U[sigma,(k,n,d)] * v, subgroup gl
            of GL chunks. fp32 throughout (fp16 anywhere in the logit path
            fails: measured 2.3e-2..0.19 rel err vs the 2e-2 gate).
            Multiply on DVE; pairwise-add d-tree in place inside pr with the
            wide rounds on GPSIMD (otherwise idle), tail + L add on DVE."""
            Lv = L.rearrange("p (a x) -> p a x", x=GL * N_CAP)
            pr = pprod.tile([128, GL * ND], F32, tag="pr")
            nc.vector.tensor_tensor(
                pr.rearrange("p (k f) -> p k f", k=GL),
                U[:, gli * GL * ND:(gli + 1) * GL * ND]
                    .rearrange("p (k f) -> p k f", k=GL),
                vf[:].unsqueeze(1).to_broadcast([128, GL, ND]),
                ALU.mult,
            )
            pv = pr.rearrange("p (a d) -> p a d", d=D_CAP)  # a = GL*N_CAP
            if tree_reduce:
                red = pprod.tile([128, GL * N_CAP], F32, tag="red")
                nc.vector.tensor_reduce(red[:], pv, axis=mybir.AxisListType.X,
                                        op=ALU.add)
                nc.vector.tensor_tensor(Lv[:, gli, :], Lv[:, gli, :], red[:],
                                        ALU.add)
            else:
                teng = nc.gpsimd if use_pool else nc.vector
                for w in (8, 4, 2):
                    teng.tensor_tensor(
                        pv[:, :, 0:w], pv[:, :, 0:w], pv[:, :, w:2 * w],
                        ALU.add)
                nc.vector.tensor_tensor(
                    pv[:, :, 0:1], pv[:, :, 0:1], pv[:, :, 1:2], ALU.add)
                nc.vector.tensor_tensor(
                    Lv[:, gli, :], Lv[:, gli, :],
                    pv[:, :, 0:1].rearrange("p a d -> p (a d)"),
                    ALU.add)

        def softmax_smm_group(g, s_ps):
            """Group-local softmax + C_all build + the s accumulation MMs."""
            Lg = L.rearrange("p (g k n) -> p g k n", k=GRP, n=N_CAP)
            ee = psm.tile([128, GRP * N_CAP], F32, tag="ee")
            nc.scalar.activation(
                ee[:], Lg[:, g].rearrange("p k n -> p (k n)"), AF.Exp)
            eev = ee.rearrange("p (k n) -> p k n", n=N_CAP)
            den = psm.tile([128, GRP], F32, tag="den")
            nc.vector.tensor_reduce(
                den[:], eev, axis=mybir.AxisListType.X, op=ALU.add)
            rr = psm.tile([128, GRP], F32, tag="rr")
            nc.vector.reciprocal(rr[:], den[:])
            cc = psm.tile([128, GRP * N_CAP], F32, tag="cc")
            nc.vector.tensor_tensor(
                cc.rearrange("p (k n) -> p k n", n=N_CAP), eev,
                rr[:].unsqueeze(2).to_broadcast([128, GRP, N_CAP]),
                ALU.mult,
            )
            ca = pca.tile([128, GRP * 80], F32)
            nc.vector.tensor_tensor(
                ca.rearrange("p (k n b) -> p k n b", k=GRP, b=8),
                cc.rearrange("p (k n) -> p k n", n=N_CAP)
                    .unsqueeze(3).to_broadcast([128, GRP, N_CAP, 8]),
                maskc_sb.rearrange("p (n b) -> p n b", b=8)
                    .unsqueeze(1).to_broadcast([128, GRP, N_CAP, 8]),
                ALU.mult,
            )
            for kk in range(GRP):
                k = g * GRP + kk
                nc.tensor.matmul(
                    s_ps[:],
                    ca[:, kk * 80:(kk + 1) * 80],
                    U[:, k * ND:(k + 1) * ND],
                    start=(k == 0), stop=(k == CH - 1),
                )

        def s_iteration(v_prev=None):
            """If v_prev is given, fuse its logit-update per group with this
            iteration's softmax + s-matmuls (the PE chases the DVE)."""
            vf = broadcast_v(v_prev) if v_prev is not None else None
            s_ps = ppsS.tile([80, ND], F32, tag="s_acc")
            sub = GRP // GL
            fill_ps = None
            if fillers > 0:
                fill_ps = ppsF.tile([8, ND], F32, tag="fill")
            for g in range(NG):
                if vf is not None:
                    for q in range(sub):
                        logit_update_group(g * sub + q, vf)
                softmax_smm_group(g, s_ps)
                # PE keep-warm fillers: the DVE/POOL chain for group g+1 takes
                # ~7us while the PE burst is ~5us; dummy matmuls bridge the gap
                # so the PE p-state stays at peak (idle >100ns drops matmuls to
                # half rate for the next 3us of execution)
                for _ in range(fillers):
                    nc.tensor.matmul(fill_ps[:], mask0_sb[:], maskd_sb[:],
                                     start=True, stop=True)
            return s_ps

        def extract_diag(s_ps):
            """[80,160] psum -> [8,160]: mask away the n-offdiagonal blocks,
            then one selection matmul folds (n*8+b) rows onto b rows."""
            sm = psmall.tile([80, ND], F32, tag="vf")
            nc.vector.tensor_tensor(sm[:], s_ps[:], maskd_sb[0:80, :],
                                    ALU.mult)
            s8_ps = ppsS.tile([8, ND], F32, tag="s_ex")
            nc.tensor.matmul(s8_ps[:], selb_sb[0:80, :], sm[:], start=True,
                             stop=True)
            s_sb = psmall.tile([8, ND], F32, tag="sq_s")
            nc.scalar.copy(s_sb[:], s8_ps[:])
            return s_sb

        # ---- routing (logit-update of iter t fused into s-iteration t+1) ----
        for rep in range(repeat):
            if rep > 0:
                nc.vector.memset(L[:], 0.0)
            s0raw = produce()
            sc0 = 0.1 if s0_pe else 1.0
            if phases in ("prod", "proddbg"):
                v_sb = squash(s_uniform(s0raw)[:], sc0)
            elif phases == "it1":
                v_sb = squash(s_uniform(s0raw)[:], sc0)
                v_sb = squash(extract_diag(s_iteration(v_sb))[:], 1.0,
                              copy_in=False)
            else:
                # the 1/N is baked into mask0 (or applied via sc0)
                v_sb = squash(s_uniform(s0raw)[:], sc0)
                for it in range(1, ROUTING_ITERS):
                    s_sb = extract_diag(s_iteration(v_sb))
                    v_sb = squash(s_sb[:], 1.0, copy_in=False)

            nc.sync.dma_start(out_d[:], v_sb[:])
            if udbg_d is not None:
                nc.sync.dma_start(udbg_d[:], U[:])
                for j in range(4):
                    nc.sync.dma_start(
                        adbg_d[:, j * ND:(j + 1) * ND], accs[j][:])

    nc.compile()
    if not nc.is_finalized():
        nc.finalize()
    return nc


# ------------------------- host-side data prep -------------------------

def prep_core_inputs(x_shard, I_dim=I_FULL, fp16_produce=False):
    """Per-core xpk (fp16 hi/lo pair or fp32) from x_shard [8, I, 8] f32."""
    CH = I_dim // 16
    b_shard = x_shard.shape[0]
    assert b_shard == 8

    # xs[b, k, i', p] -> xpk[(i'*8+p), k*8+b]
    xs = x_shard.reshape(b_shard, CH, 16, P_DIM)
    xpk = np.ascontiguousarray(
        np.transpose(xs, (2, 3, 1, 0)).reshape(128, CH * 8))
    if not fp16_produce:
        return {"xpkp": xpk}
    xh = xpk.astype(np.float16)
    xl = (xpk - xh.astype(np.float32)).astype(np.float16)
    return {"xpkp": np.ascontiguousarray(np.concatenate([xh, xl], axis=1))}


def prep_shared_inputs(W_np, fp16_produce=False):
    # w2[(i*8+p), n*16+d] = W[i, n, p, d]; fp16 hi/lo pair or plain fp32
    w2 = np.ascontiguousarray(
        np.transpose(W_np, (0, 2, 1, 3)).reshape(-1, ND).astype(np.float32))
    if fp16_produce:
        w2h = w2.astype(np.float16)
        w2l = (w2 - w2h.astype(np.float32)).astype(np.float16)
        w2p = np.ascontiguousarray(np.concatenate([w2h, w2l], axis=1))
    else:
        w2p = w2

    # mask0[sigma, b'] = 0.1 * (b' == b(sigma));  sigma = i'*8+b
    # maskc[sigma, n*8+b'] = (b' == b(sigma)); s-matmul output partition = n*8+b
    # xmask[q=(i'*8+p), i2*8+b] = (i2 == i')
    mask0 = np.zeros((128, 8), dtype=np.float32)
    maskc = np.zeros((128, 80), dtype=np.float32)
    e8 = np.zeros((8, 128), dtype=np.float32)
    xmask = np.zeros((128, 128),
                     dtype=np.float16 if fp16_produce else np.float32)
    maskd = np.zeros((128, ND), dtype=np.float32)
    selb = np.zeros((128, 8), dtype=np.float32)
    for ip in range(16):
        for b in range(8):
            sig = ip * 8 + b
            mask0[sig, b] = 0.1
            maskc[sig, b::8] = 1.0
            e8[b, sig] = 1.0
    for ii in range(16):
        for p in range(P_DIM):
            xmask[ii * 8 + p, ii * 8:(ii + 1) * 8] = 1.0
    for n in range(N_CAP):
        for b in range(8):
            maskd[n * 8 + b, n * D_CAP:(n + 1) * D_CAP] = 1.0
            selb[n * 8 + b, b] = 1.0
    return {"w2p": w2p, "mask0": mask0, "maskc": maskc, "e8": e8,
            "xmask": xmask, "maskd": maskd, "selb": selb}


_NC_CACHE = {}
LAST_RESULT = None  # BassKernelResults of the most recent kernel() call


def _get_nc(I_dim=I_FULL):
    if I_dim not in _NC_CACHE:
        _NC_CACHE[I_dim] = build_nc(I_dim)
    return _NC_CACHE[I_dim]


def kernel(inputs: np.ndarray, W: np.ndarray, trace: bool = False) -> np.ndarray:
    global LAST_RESULT
    from concourse.bass_utils import run_bass_kernel_spmd

    inputs = np.asarray(inputs, dtype=np.float32)
    W = np.asarray(W, dtype=np.float32)
    B, I_dim, _ = inputs.shape

    nc = _get_nc(I_dim)
    shared = prep_shared_inputs(W)

    in_maps = []
    bs = B // N_CORES
    for c in range(N_CORES):
        m = dict(shared)
        m.update(prep_core_inputs(inputs[c * bs:(c + 1) * bs], I_dim))
        in_maps.append(m)

    res = run_bass_kernel_spmd(nc, in_maps, list(range(N_CORES)), trace=trace)
    LAST_RESULT = res
    outs = [res.results[c]["out"] for c in range(N_CORES)]
    v = np.concatenate(outs, axis=0)          # [64, 160]
    v = v.reshape(B, N_CAP, D_CAP)[:, :, None, :]   # [64, 10, 1, 16]
    return v.astype(np.float32)
